# revision 26
# baseline (speedup 1.0000x reference)
"""Causal self-attention kernel for Trainium2, data-parallel over batch on 8 cores.

Reference computation (B=256, T=256, C=192, H=6, D=32):
    qkv = x @ w_qkv.T -> q,k,v ; scores = q k^T / sqrt(D) causal-masked
    y = softmax(scores) @ v ; out = y @ w_out.T

Per-core design (32 batches, fp16 matmul operands, fp32 accumulation),
3-stage software pipeline so the PE never waits on the q/k shuffle DMA:

  iteration i:   S1(i): x load(i+1), x^T, qkv matmuls + casts, merged
                        one-instruction shuffle DMA -> [32, 3072] q^T/k^T
                 S2(i-1): per-head scores -> exp (ScalarE, scale folded) ->
                        causal mask (gpsimd affine_select, diagonal 128-blocks
                        only, in place) -> rowsum + attn@v matmuls ->
                        reciprocal + multiply -> y^T fp16
                 S3(i-2): out = y @ w_out.T -> DRAM

  Scores for head h are spread between the qkv matmuls of the next batch so
  the ScalarE exp chain keeps pace with a 3-deep score-tile ring.

  PSUM (8 banks): scores ring 3 + qk/out shared slot (disjoint lifetimes,
  bufs=1) 2 + v 1 + rowsums/x^T shared 1 + attn@v 1.
"""
import sys

sys.path.insert(0, "/opt/trn_rl_repo")

import numpy as np

B, T, C, H, D = 256, 256, 192, 6, 32
NCORES = 8
BPC = B // NCORES  # 32 batches per core
SCALE = 1.0 / np.sqrt(np.float32(D))

# column offsets of each head's q^T / k^T block inside the [32, 3072] shuffle
QCOL = [0, 768, 1536, 2304, 256, 1024]
KCOL = [1792, 2560, 512, 1280, 2048, 2816]

_CACHE = {}


def _build(bpc=BPC, repeat=0, o_copy_engine="scalar"):
    import contextlib
    from concourse import bacc, tile, mybir
    from concourse.masks import make_identity

    F32 = mybir.dt.float32
    F16 = mybir.dt.float16
    Exp = mybir.ActivationFunctionType.Exp

    nc = bacc.Bacc(None, target_bir_lowering=False)
    x_d = nc.dram_tensor("x", [bpc, T, C], F32, kind="ExternalInput")
    wqkv_d = nc.dram_tensor("w_qkv", [3 * C, C], F32, kind="ExternalInput")
    wout_d = nc.dram_tensor("w_out", [C, C], F32, kind="ExternalInput")
    out_d = nc.dram_tensor("out", [bpc, T, C], F32, kind="ExternalOutput")

    with tile.TileContext(nc) as tc:
        with tc.tile_pool(name="cst", bufs=1) as cst, \
             tc.tile_pool(name="sb", bufs=2) as sb, \
             tc.tile_pool(name="ps", bufs=1, space="PSUM") as ps:
            ident = cst.tile([128, 128], F32)
            make_identity(nc, ident[:])
            ident16 = cst.tile([128, 128], F16)
            nc.vector.tensor_copy(ident16[:], ident[:])
            ones16 = cst.tile([128, 32], F16)
            nc.vector.memset(ones16[:], 1.0)

            # ---- one-time: transpose w_qkv -> wqT fp16 [2][96, 576] ----
            wq_sb = cst.tile([128, 5, 192], F32)
            wq_v = wqkv_d[0:512, :].rearrange("(n p) c -> p n c", p=128)
            nc.sync.dma_start(wq_sb[:, 0:4, :], wq_v)
            nc.sync.dma_start(wq_sb[0:64, 4, :], wqkv_d[512:576, :])
            wqT = []
            for cb in range(2):
                w16 = cst.tile([96, 576], F16, name=f"wqT{cb}")
                for ot in range(5):
                    rows = 128 if ot < 4 else 64
                    wt_ps = ps.tile([96, 128], F32, tag="sp", bufs=3,
                                    name=f"wt{cb}_{ot}")
                    nc.tensor.transpose(
                        wt_ps[:, 0:rows],
                        wq_sb[0:rows, ot, cb * 96:cb * 96 + 96],
                        ident[0:rows, 0:rows])
                    nc.vector.tensor_copy(
                        w16[:, ot * 128:ot * 128 + rows], wt_ps[:, 0:rows])
                wqT.append(w16)

            # ---- one-time: transpose w_out -> woT fp16 [128,192]+[64,192] ----
            wo_sb = cst.tile([128, 2, 192], F32)
            nc.sync.dma_start(wo_sb[:, 0, :], wout_d[0:128, :])
            nc.sync.dma_start(wo_sb[0:64, 1, :], wout_d[128:192, :])
            woT = []
            for cb, (p0, rows) in enumerate([(0, 128), (128, 64)]):
                w16 = cst.tile([rows, 192], F16, name=f"woT{cb}")
                for ot, (q0, cols) in enumerate([(0, 128), (128, 64)]):
                    wt_ps = ps.tile([128, 128], F32, tag="sp", bufs=3,
                                    name=f"wo{cb}_{ot}")
                    nc.tensor.transpose(
                        wt_ps[0:rows, 0:cols],
                        wo_sb[0:cols, ot, p0:p0 + rows],
                        ident[0:cols, 0:cols])
                    nc.vector.tensor_copy(
                        w16[:, ot * 128:ot * 128 + cols], wt_ps[0:rows, 0:cols])
                woT.append(w16)

            x_v = x_d.rearrange("b (u p) c -> b p u c", p=128)
            o_v = out_d.rearrange("b (u p) c -> b p u c", p=128)

            st = {}  # live per-batch tiles

            def load(i):
                x_sb = sb.tile([128, 2, 192], F32, tag="x", name=f"x{i}")
                nc.sync.dma_start(x_sb[:], x_v[i])
                st[("x", i)] = x_sb

            def prep(i):
                # x cast fp16 + PE transpose -> x^T [96, 2x256] in SBUF
                x16 = sb.tile([128, 2, 192], F16, tag="x16", name=f"x16_{i}")
                nc.vector.tensor_copy(x16[:], st.pop(("x", i))[:])
                xt_ps = ps.tile([96, 512], F16, tag="vx", bufs=1,
                                name=f"xt{i}")
                for u in range(2):
                    for cb in range(2):
                        nc.tensor.transpose(
                            xt_ps[:, cb * 256 + u * 128:cb * 256 + u * 128 + 128],
                            x16[:, u, cb * 96:cb * 96 + 96], ident16[:])
                xt16 = sb.tile([96, 512], F16, tag="xt16", name=f"xt16_{i}")
                nc.vector.tensor_copy(xt16[:], xt_ps[:])
                st[("xt", i)] = xt16

            def qk_mm(i, ot):
                if ot == 0:
                    st[("qkps", i)] = ps.tile([128, 768], F32, tag="qo",
                                              bufs=1, name=f"qk{i}")
                qk_ps, xt16 = st[("qkps", i)], st[("xt", i)]
                for cb in range(2):
                    nc.tensor.matmul(
                        qk_ps[:, ot * 256:ot * 256 + 256],
                        wqT[cb][:, ot * 128:ot * 128 + 128],
                        xt16[:, cb * 256:cb * 256 + 256],
                        start=(cb == 0), stop=(cb == 1))

            def qk_fin(i):
                # cast fp16 + merged one-instruction shuffle -> [32, 3072]
                qk16 = sb.tile([128, 768], F16, tag="qk16", name=f"qk16_{i}")
                nc.vector.tensor_copy(qk16[:], st.pop(("qkps", i))[:])
                qkT32 = sb.tile([32, 3072], F16, tag="qkT32", name=f"qkT32_{i}")
                for g in range(4):
                    nc.sync.dma_start(qkT32[0:32, g * 768:(g + 1) * 768],
                                      qk16[32 * g:32 * g + 32, :])
                st[("qkT", i)] = qkT32

            def v_mm(i, u):
                if u == 0:
                    st[("vps", i)] = ps.tile([128, 384], F32, tag="vx",
                                             bufs=1, name=f"v{i}")
                v_ps, xt16 = st[("vps", i)], st[("xt", i)]
                for cb in range(2):
                    nc.tensor.matmul(
                        v_ps[:, u * 192:u * 192 + 192],
                        xt16[:, cb * 256 + u * 128:cb * 256 + u * 128 + 128],
                        wqT[cb][:, 384:576],
                        start=(cb == 0), stop=(cb == 1))

            def v_fin(i):
                st.pop(("xt", i))
                v16 = sb.tile([128, 2, 192], F16, tag="v16", name=f"v16_{i}")
                nc.vector.tensor_copy(v16[:], st.pop(("vps", i))[:])
                st[("v16", i)] = v16

            def score_head(i, h):
                # S^T per head: k0 rows t_k 0:128 (cols 0:256 = t_q), k1 rows
                # t_k 128:256 (cols 256:384 = t_q 128:256); exp with folded
                # 1/sqrt(D); causal zero-fill only on the two diagonal blocks.
                qkT32 = st[("qkT", i)]
                qc, kc = QCOL[h], KCOL[h]
                sp = ps.tile([128, 384], F32, tag="sp", bufs=3,
                             name=f"s{h}_{i}")
                nc.tensor.matmul(
                    sp[:, 0:256], qkT32[0:32, kc:kc + 128],
                    qkT32[0:32, qc:qc + 256],
                    start=True, stop=True, tile_position=(0, 0))
                nc.tensor.matmul(
                    sp[:, 256:384], qkT32[0:32, kc + 128:kc + 256],
                    qkT32[0:32, qc + 128:qc + 256],
                    start=True, stop=True, tile_position=(0, 0))
                ap = sb.tile([128, 512], F16, tag=f"ax{h}", name=f"ax{h}_{i}")
                nc.scalar.activation(ap[:, 0:384], sp[:], Exp,
                                     scale=float(SCALE))
                apm = sb.tile([128, 512], F16, tag=f"at{h}", name=f"at{h}_{i}")
                a_in = ap[:].rearrange("p (g q) -> p g q", g=2)
                a_out = apm[:].rearrange("p (g q) -> p g q", g=2)
                nc.gpsimd.affine_select(
                    out=a_out, in_=a_in, compare_op=mybir.AluOpType.is_ge,
                    fill=0.0, base=0, pattern=[[0, 2], [1, 256]],
                    channel_multiplier=-1)
                st[("at", i, h)] = apm

            def sums_av(i):
                sums_ps = ps.tile([128, 512], F32, tag="sm", bufs=1,
                                  name=f"sm{i}")
                av_ps = ps.tile([128, 512], F32, tag="av", bufs=1,
                                name=f"av{i}")
                v16 = st.pop(("v16", i))
                # per-head, strictly k0-then-k1 order: start=True marks the
                # whole 2KB zero-region pending, so heads sharing partitions
                # must fully finish before the next starts.
                for h in range(6):
                    pb, cc = 32 * (h % 4), 256 * (h // 4)
                    am = st.pop(("at", i, h))
                    k0, k1 = am[:, 0:256], am[:, 256:384]
                    nc.tensor.matmul(
                        sums_ps[pb:pb + 32, cc:cc + 256], ones16[:], k0,
                        start=True, stop=False, tile_position=(0, pb),
                        skip_group_check=True)
                    nc.tensor.matmul(
                        sums_ps[pb:pb + 32, cc + 128:cc + 256], ones16[:], k1,
                        start=False, stop=True, tile_position=(0, pb),
                        skip_group_check=True)
                    nc.tensor.matmul(
                        av_ps[pb:pb + 32, cc:cc + 256],
                        v16[:, 0, h * 32:h * 32 + 32], k0,
                        start=True, stop=False, tile_position=(0, pb),
                        skip_group_check=True)
                    nc.tensor.matmul(
                        av_ps[pb:pb + 32, cc + 128:cc + 256],
                        v16[:, 1, h * 32:h * 32 + 32], k1,
                        start=False, stop=True, tile_position=(0, pb),
                        skip_group_check=True)
                st[("sums", i)] = sums_ps
                st[("avps", i)] = av_ps

            def normalize(i):
                sums_ps = st.pop(("sums", i))
                av_ps = st.pop(("avps", i))
                recip = sb.tile([128, 512], F32, tag="rc", name=f"rc{i}")
                nc.vector.reciprocal(recip[:, 0:256], sums_ps[:, 0:256])
                nc.vector.reciprocal(recip[0:64, 256:512],
                                     sums_ps[0:64, 256:512])
                yT0 = sb.tile([128, 256], F16, tag="yT0", bufs=3,
                              name=f"yT0_{i}")
                yT1 = sb.tile([64, 256], F16, tag="yT1", bufs=3,
                              name=f"yT1_{i}")
                nc.vector.tensor_mul(yT0[:], av_ps[:, 0:256], recip[:, 0:256])
                nc.vector.tensor_mul(yT1[:], av_ps[0:64, 256:512],
                                     recip[0:64, 256:512])
                st[("y", i)] = (yT0, yT1)

            def out_proj(i):
                yT0, yT1 = st.pop(("y", i))
                o_ps = ps.tile([128, 384], F32, tag="qo", bufs=1,
                               name=f"o{i}")
                for u in range(2):
                    nc.tensor.matmul(
                        o_ps[:, u * 192:u * 192 + 192],
                        yT0[:, u * 128:u * 128 + 128], woT[0][:],
                        start=True, stop=False)
                    nc.tensor.matmul(
                        o_ps[:, u * 192:u * 192 + 192],
                        yT1[:, u * 128:u * 128 + 128], woT[1][:],
                        start=False, stop=True)
                o_sb = sb.tile([128, 2, 192], F32, tag="ob", name=f"ob{i}")
                if o_copy_engine == "gpsimd":
                    nc.gpsimd.tensor_copy(o_sb[:], o_ps[:])
                else:
                    nc.scalar.copy(o_sb[:], o_ps[:])
                nc.sync.dma_start(o_v[i], o_sb[:])

            def body():
                load(0)
                prep(0)
                for i in range(bpc + 3):
                    if i + 1 < bpc:
                        load(i + 1)
                    # out(i-3) first: its inputs (DVE muls) are two
                    # iterations old, and both directions of its shared-slot
                    # WAR point at the previous iteration's work
                    if 0 <= i - 3:
                        out_proj(i - 3)
                    sc = 0 <= i - 1 < bpc
                    s1 = i < bpc
                    # scores spread between next batch's qkv matmuls so the
                    # ScalarE exp chain keeps pace with the 3-slot score ring
                    if sc:
                        score_head(i - 1, 0)
                        score_head(i - 1, 1)
                    if s1:
                        qk_mm(i, 0)
                        qk_mm(i, 1)
                    if sc:
                        score_head(i - 1, 2)
                    if s1:
                        qk_mm(i, 2)
                        qk_fin(i)
                    if sc:
                        score_head(i - 1, 3)
                    if s1:
                        v_mm(i, 0)
                    if sc:
                        score_head(i - 1, 4)
                    if s1:
                        v_mm(i, 1)
                    if sc:
                        score_head(i - 1, 5)
                    if s1:
                        v_fin(i)
                    if sc:
                        sums_av(i - 1)
                        normalize(i - 1)
                    if i + 1 < bpc:
                        prep(i + 1)

            if repeat:
                with tc.For_i(0, repeat):
                    body()
            else:
                body()

    nc.compile()
    return nc


def _get_nc():
    if "nc" not in _CACHE:
        _CACHE["nc"] = _build()
    return _CACHE["nc"]


def kernel(x: np.ndarray, w_qkv: np.ndarray, w_out: np.ndarray) -> np.ndarray:
    from concourse.bass_utils import run_bass_kernel_spmd

    nc = _get_nc()
    x = np.ascontiguousarray(np.asarray(x, dtype=np.float32))
    w_qkv = np.ascontiguousarray(np.asarray(w_qkv, dtype=np.float32))
    w_out = np.ascontiguousarray(np.asarray(w_out, dtype=np.float32))
    in_maps = [
        {"x": x[i * BPC:(i + 1) * BPC], "w_qkv": w_qkv, "w_out": w_out}
        for i in range(NCORES)
    ]
    res = run_bass_kernel_spmd(nc, in_maps, core_ids=list(range(NCORES)))
    out = np.concatenate([r["out"] for r in res.results], axis=0)
    return out.astype(np.float32)
